# revision 14
# baseline (speedup 1.0000x reference)
"""Expert-parallel MoE SwiGLU FFN kernel for 8 Trainium2 NeuronCores.

Problem: T=4096 tokens, DIM=1024, E=8 experts, INTER=1408, top-2 routing.
Reference computes all experts densely then gathers; we instead route on the
host (sort token-slots by expert), assign one expert per core, and each core
runs a SwiGLU FFN over only its routed tokens (padded to a common capacity so
all 8 cores execute the same SPMD program).

Device layout (per core, everything "transposed" with tokens on the free dim):
  xt[j]  [P, KT, n]        bf16  x_gathered.T for chunk j (part, k-tile, token)
  w13t   [P, MT, 2, KT*P]  bf16  w1/w3 interleaved per m-column, p-major so
                                 any m-slice is per-partition contiguous
  w2t    [P, KT, MT*P]     bf16  w2.T, p-major, grouped by output k-tile so
                                 phase B can start once half has landed
  yt     [KT, P, C]        bf16  y.T tiled over DIM (output; host upcasts)

Compute per core and token chunk (n <= 512 for PSUM):
  h1.T = w1 @ x.T, h3.T = w3 @ x.T   interleaved per k so the head DMA feed
                                      of x k-tiles is half rate -> PSUM
  g.T  = silu(h1.T) * h3.T           -> SBUF bf16
  y.T  = w2 @ g.T                    -> PSUM -> SBUF bf16 -> HBM

Schedule notes (from perfetto traces): the matmul stream runs at the bf16
streaming roofline (LDWEIGHTS hidden), so the binding constraints are at the
edges. The head is DMA-bandwidth-bound (~370 GB/s/core aggregate), so input
transfers are issued in strict need-time order across the three trigger
queues (sync + scalar HWDGE, gpsimd SWDGE) with tiny first pieces; dummy
warmup matmuls during the DMA head flip the HAM clock gate (the PE otherwise
runs at 1.2 GHz for its first ~3.4us of activity).
"""

import numpy as np
import ml_dtypes

T, DIM, E, INTER, TOPK = 4096, 1024, 8, 1408, 2
NCORES = 8
P = 128
KT = DIM // P    # 8 k-tiles over DIM
MT = INTER // P  # 11 m-tiles over INTER

WARM_N = 64      # dummy warmup matmuls (N=64 each) to flip the HAM clock gate

TRACE = False  # test.py sets this to capture an NTFF profile
LAST_RESULTS = None  # BassKernelResults of the last run (for test.py)

_NC_CACHE = {}


def _chunks_for(C):
    # Split C into chunks of at most 512 (PSUM bank = 512 fp32), multiples of
    # 16, all >= 256 so matmuls stay stream-bound (LDWEIGHTS ~97ns hides only
    # when N >= ~233), smallest chunk first so compute starts earliest.
    nch = -(-C // 512)
    out = []
    rem = C
    for i in range(nch, 0, -1):
        n = min(512, -(-rem // i))
        n = -(-n // 16) * 16 if i > 1 else rem  # keep multiples of 16
        n = min(n, 512, rem)
        out.append(n)
        rem -= n
    out.sort()
    assert sum(out) == C and all(0 < n <= 512 for n in out), out
    return out


def _build_nc(C):
    import concourse.mybir as mybir
    import concourse.tile as tile
    from concourse import bacc

    dt = mybir.dt
    AF = mybir.ActivationFunctionType
    chunks = _chunks_for(C)

    nc = bacc.Bacc(
        "TRN2", target_bir_lowering=False, debug=False, enable_asserts=False
    )
    xts = [
        nc.dram_tensor(f"xt{j}", [P, KT, n], dt.bfloat16, kind="ExternalInput")
        for j, n in enumerate(chunks)
    ]
    w13t = nc.dram_tensor("w13t", [P, MT, KT, 2, P], dt.bfloat16, kind="ExternalInput")
    w2t = nc.dram_tensor("w2t", [P, KT, MT * P], dt.bfloat16, kind="ExternalInput")
    yt = nc.dram_tensor("yt", [KT, P, C], dt.bfloat16, kind="ExternalOutput")

    with tile.TileContext(nc) as tc:
        with (
            tc.tile_pool(name="persist", bufs=1) as wpool,
            tc.tile_pool(name="gbuf", bufs=3) as gpool,
            tc.tile_pool(name="ybuf", bufs=4) as ypool,
            tc.tile_pool(name="silbuf", bufs=3) as spool,
            tc.tile_pool(name="psA", bufs=2, space="PSUM") as psA,
            tc.tile_pool(name="psB", bufs=2, space="PSUM") as psB,
            tc.tile_pool(name="psW", bufs=1, space="PSUM") as psW,
        ):
            xss = [wpool.tile([P, KT, n], dt.bfloat16, name=f"xs{j}")
                   for j, n in enumerate(chunks)]
            w13s = wpool.tile([P, MT, KT, 2, P], dt.bfloat16)
            w2s = wpool.tile([P, KT, MT * P], dt.bfloat16)
            scratch = wpool.tile([P, 64], dt.bfloat16, name="warm")

            # Input DMAs in strict need-time order; the head is DMA-BW bound,
            # so nothing may run ahead of more urgently needed bytes, and the
            # crunch set (x chunk0 + w13 m0/m1) is balanced across the three
            # trigger queues (each trigger costs ~0.6us of sequencer time; a
            # queue's transfers complete in ring order).
            # Deadline-critical bytes go on the two HWDGE rings only (the
            # gpsimd SWDGE ring is slower): scalar takes the first weight
            # columns + x tail, sync takes the x head + remaining columns.
            nc.scalar.dma_start(w13s[:, 0, 0:2], w13t[:, 0, 0:2])
            nc.scalar.dma_start(w13s[:, 0, 2:], w13t[:, 0, 2:])
            nc.scalar.dma_start(w13s[:, 1], w13t[:, 1])
            nc.scalar.dma_start(xss[0][:, 4:, :], xts[0][:, 4:, :])
            # sync: x chunk0 head, weight columns m2.. (y outputs are appended
            # to this ring later, in program order).
            nc.sync.dma_start(xss[0][:, 0:2, :], xts[0][:, 0:2, :])
            nc.sync.dma_start(xss[0][:, 2:4, :], xts[0][:, 2:4, :])
            nc.sync.dma_start(w13s[:, 2], w13t[:, 2])
            nc.sync.dma_start(w13s[:, 3], w13t[:, 3])
            nc.sync.dma_start(w13s[:, 4:7], w13t[:, 4:7])
            nc.sync.dma_start(w13s[:, 7:], w13t[:, 7:])
            # gpsimd (SWDGE): only bytes with late deadlines.
            nc.gpsimd.dma_start(w2s[:, 0:4], w2t[:, 0:4])
            nc.gpsimd.dma_start(w2s[:, 4:], w2t[:, 4:])
            for j in range(1, len(chunks)):
                nc.gpsimd.dma_start(xss[j][:], xts[j][:])

            # Warmup: tiny matmuls on a scratch tile keep the PE busy during
            # the DMA head so the HAM clock gate is at 8/8 when real work lands.
            nc.vector.memset(scratch[:], 0.0)
            wps = psW.tile([64, 64], dt.float32, name="warmps")
            for _ in range(WARM_N):
                nc.tensor.matmul(wps[:], scratch[:, 0:64], scratch[:], start=True, stop=True)

            c0 = 0
            for j, n in enumerate(chunks):
                xsj = xss[j]
                gs = gpool.tile([P, MT, n], dt.bfloat16, name="gs")
                for m in range(MT):
                    p1 = psA.tile([P, n], dt.float32, name="p1")
                    p3 = psA.tile([P, n], dt.float32, name="p3")
                    # Interleave the w1/w3 k-loops: halves the x k-tile
                    # consumption rate while the head DMAs are still landing.
                    for k in range(KT):
                        nc.tensor.matmul(
                            p1[:],
                            w13s[:, m, k, 0, :],
                            xsj[:, k, :],
                            start=(k == 0),
                            stop=(k == KT - 1),
                        )
                        nc.tensor.matmul(
                            p3[:],
                            w13s[:, m, k, 1, :],
                            xsj[:, k, :],
                            start=(k == 0),
                            stop=(k == KT - 1),
                        )
                    sil = spool.tile([P, n], dt.bfloat16, name="sil")
                    nc.scalar.activation(sil[:], p1[:], AF.Silu)
                    nc.vector.tensor_mul(gs[:, m, :], sil[:], p3[:])
                for i in range(KT):
                    py = psB.tile([P, n], dt.float32, name="py")
                    for m in range(MT):
                        nc.tensor.matmul(
                            py[:],
                            w2s[:, i, m * P:(m + 1) * P],
                            gs[:, m, :],
                            start=(m == 0),
                            stop=(m == MT - 1),
                        )
                    ys = ypool.tile([P, n], dt.bfloat16, name="ys")
                    nc.vector.tensor_copy(ys[:], py[:])
                    nc.sync.dma_start(yt[i, :, c0:c0 + n], ys[:])
                c0 += n

    nc.compile()
    return nc


def _get_nc(C):
    if C not in _NC_CACHE:
        _NC_CACHE[C] = _build_nc(C)
    return _NC_CACHE[C]


def _ensure_ntff_hook_importable():
    # bass_utils imports antenv.axon_hooks when tracing is requested (e.g. via
    # a BASS_TRACE env var); in containers whose antenv stub lacks that
    # submodule the import would crash. Register a null hook so tracing just
    # degrades to "no trace" instead.
    import sys
    import types

    try:
        import antenv.axon_hooks  # noqa: F401
    except ImportError:
        mod = types.ModuleType("antenv.axon_hooks")
        mod.get_axon_ntff_profile_hook = lambda: None
        mod.set_axon_ntff_profile_hook = lambda hook: None
        sys.modules["antenv.axon_hooks"] = mod


def kernel(x, expert_indices, w1, w2, w3):
    global LAST_RESULTS
    import os
    import sys

    # The bass kernel executes on the NeuronCores via the axon PJRT backend;
    # a JAX_PLATFORMS=cpu pin (commonly used for running jax reference code)
    # would hide those devices. Clear it if jax hasn't initialized yet.
    if os.environ.get("JAX_PLATFORMS") == "cpu" and "jax" not in sys.modules:
        del os.environ["JAX_PLATFORMS"]

    from concourse import bass_utils

    _ensure_ntff_hook_importable()
    x = np.asarray(x, dtype=np.float32)
    idx = np.asarray(expert_indices)
    w1 = np.asarray(w1, dtype=np.float32)
    w2 = np.asarray(w2, dtype=np.float32)
    w3 = np.asarray(w3, dtype=np.float32)

    bf16 = ml_dtypes.bfloat16

    # --- host routing: stable-sort the (token, k) slots by expert id ---
    flat = idx.reshape(-1).astype(np.int64)  # slot s = t*TOPK + k -> expert
    order = np.argsort(flat, kind="stable")  # slots grouped by expert
    counts = np.bincount(flat, minlength=E)
    starts = np.zeros(E + 1, dtype=np.int64)
    np.cumsum(counts, out=starts[1:])
    cmax = int(counts.max())
    C = max(512, -(-cmax // 16) * 16)  # pad capacity to a multiple of 16

    nc = _get_nc(C)

    chunks = _chunks_for(C)
    bounds = np.cumsum([0] + chunks)
    xb = x.astype(bf16)
    in_maps = []
    for e in range(E):
        slots = order[starts[e]:starts[e + 1]]
        tokens = slots // TOPK
        xg = np.zeros((C, DIM), dtype=bf16)
        xg[: len(tokens)] = xb[tokens]
        # [C, DIM] -> [P, KT, C] (partition-major), then per-chunk blocks
        xpkc = xg.T.reshape(KT, P, C).transpose(1, 0, 2)
        im = {
            f"xt{j}": np.ascontiguousarray(xpkc[:, :, bounds[j]:bounds[j + 1]])
            for j in range(len(chunks))
        }
        # w13t[p, m, k, s, j] = (w1 if s==0 else w3)[e][m*128+j, k*128+p]
        w1r = w1[e].astype(bf16).reshape(MT, P, KT, P).transpose(3, 0, 2, 1)
        w3r = w3[e].astype(bf16).reshape(MT, P, KT, P).transpose(3, 0, 2, 1)
        im["w13t"] = np.ascontiguousarray(np.stack([w1r, w3r], axis=3))
        # w2t[p, i, m*128+j] = w2[e][i*128+j, m*128+p]
        im["w2t"] = np.ascontiguousarray(
            w2[e].astype(bf16).reshape(KT, P, MT, P).transpose(3, 0, 2, 1)
        ).reshape(P, KT, MT * P)
        in_maps.append(im)

    res = bass_utils.run_bass_kernel_spmd(
        nc, in_maps, core_ids=list(range(NCORES)), trace=TRACE
    )
    LAST_RESULTS = res

    out = np.empty((T * TOPK, DIM), dtype=np.float32)
    for e in range(E):
        slots = order[starts[e]:starts[e + 1]]
        yt = res.results[e]["yt"]  # [KT, P, C] bf16
        y = yt.reshape(DIM, C).astype(np.float32)  # y.T
        out[slots] = y[:, : len(slots)].T
    return out.reshape(T, TOPK, DIM)


# revision 16
# speedup vs baseline: 1.0671x; 1.0671x over previous
"""Expert-parallel MoE SwiGLU FFN kernel for 8 Trainium2 NeuronCores.

Problem: T=4096 tokens, DIM=1024, E=8 experts, INTER=1408, top-2 routing.
Reference computes all experts densely then gathers; we instead route on the
host (sort token-slots by expert), assign one expert per core, and each core
runs a SwiGLU FFN over only its routed tokens (padded to a common capacity so
all 8 cores execute the same SPMD program).

Device layout (per core, everything "transposed" with tokens on the free dim):
  xt[j]  [P, KT, n]        bf16  x_gathered.T for chunk j (part, k-tile, token)
  w13t   [P, MT, 2, KT*P]  bf16  w1/w3 interleaved per m-column, p-major so
                                 any m-slice is per-partition contiguous
  w2t    [P, KT, MT*P]     bf16  w2.T, p-major, grouped by output k-tile so
                                 phase B can start once half has landed
  yt     [KT, P, C]        bf16  y.T tiled over DIM (output; host upcasts)

Compute per core and token chunk (n <= 512 for PSUM):
  h1.T = w1 @ x.T, h3.T = w3 @ x.T   interleaved per k so the head DMA feed
                                      of x k-tiles is half rate -> PSUM
  g.T  = silu(h1.T) * h3.T           -> SBUF bf16
  y.T  = w2 @ g.T                    -> PSUM -> SBUF bf16 -> HBM

Schedule notes (from perfetto traces): the matmul stream runs at the bf16
streaming roofline (LDWEIGHTS hidden), so the binding constraints are at the
edges. The head is DMA-bandwidth-bound (~370 GB/s/core aggregate), so input
transfers are issued in strict need-time order across the three trigger
queues (sync + scalar HWDGE, gpsimd SWDGE) with tiny first pieces; dummy
warmup matmuls during the DMA head flip the HAM clock gate (the PE otherwise
runs at 1.2 GHz for its first ~3.4us of activity).
"""

import numpy as np
import ml_dtypes

T, DIM, E, INTER, TOPK = 4096, 1024, 8, 1408, 2
NCORES = 8
P = 128
KT = DIM // P    # 8 k-tiles over DIM
MT = INTER // P  # 11 m-tiles over INTER

WARM_N = 40      # dummy warmup matmuls (N=64 each) to flip the HAM clock gate

TRACE = False  # test.py sets this to capture an NTFF profile
LAST_RESULTS = None  # BassKernelResults of the last run (for test.py)

_NC_CACHE = {}


def _chunks_for(C):
    # Split C into chunks of at most 512 (PSUM bank = 512 fp32), multiples of
    # 16, all >= 256 so matmuls stay stream-bound (LDWEIGHTS ~97ns hides only
    # when N >= ~233), smallest chunk first so compute starts earliest.
    nch = -(-C // 512)
    out = []
    rem = C
    for i in range(nch, 0, -1):
        n = min(512, -(-rem // i))
        n = -(-n // 16) * 16 if i > 1 else rem  # keep multiples of 16
        n = min(n, 512, rem)
        out.append(n)
        rem -= n
    out.sort()
    assert sum(out) == C and all(0 < n <= 512 for n in out), out
    return out


def _build_nc(C):
    import concourse.mybir as mybir
    import concourse.tile as tile
    from concourse import bacc

    dt = mybir.dt
    AF = mybir.ActivationFunctionType
    chunks = _chunks_for(C)

    nc = bacc.Bacc(
        "TRN2", target_bir_lowering=False, debug=False, enable_asserts=False
    )
    xts = [
        nc.dram_tensor(f"xt{j}", [P, KT, n], dt.bfloat16, kind="ExternalInput")
        for j, n in enumerate(chunks)
    ]
    w13t = nc.dram_tensor("w13t", [P, MT, KT, 2, P], dt.bfloat16, kind="ExternalInput")
    w2t = nc.dram_tensor("w2t", [P, KT, MT * P], dt.bfloat16, kind="ExternalInput")
    yt = nc.dram_tensor("yt", [KT, P, C], dt.bfloat16, kind="ExternalOutput")

    with tile.TileContext(nc) as tc:
        with (
            tc.tile_pool(name="persist", bufs=1) as wpool,
            tc.tile_pool(name="gbuf", bufs=3) as gpool,
            tc.tile_pool(name="ybuf", bufs=4) as ypool,
            tc.tile_pool(name="silbuf", bufs=3) as spool,
            tc.tile_pool(name="psA", bufs=2, space="PSUM") as psA,
            tc.tile_pool(name="psB", bufs=2, space="PSUM") as psB,
            tc.tile_pool(name="psW", bufs=1, space="PSUM") as psW,
        ):
            xss = [wpool.tile([P, KT, n], dt.bfloat16, name=f"xs{j}")
                   for j, n in enumerate(chunks)]
            w13s = wpool.tile([P, MT, KT, 2, P], dt.bfloat16)
            w2s = wpool.tile([P, KT, MT * P], dt.bfloat16)
            scratch = wpool.tile([P, 64], dt.bfloat16, name="warm")

            # Input DMAs in strict need-time order; the head is DMA-BW bound,
            # so nothing may run ahead of more urgently needed bytes, and the
            # crunch set (x chunk0 + w13 m0/m1) is balanced across the three
            # trigger queues (each trigger costs ~0.6us of sequencer time; a
            # queue's transfers complete in ring order).
            # DMA engines service descriptors round-robin across rings, so a
            # transfer's effective priority is its ring position: everything
            # goes on the two HWDGE rings in deadline order (lazy bytes BEHIND
            # urgent bytes), split so both rings carry similar early loads.
            # The gpsimd SWDGE ring stays empty (its big software-generated
            # descriptors hog engine time).
            nc.scalar.dma_start(w13s[:, 0, 0:2], w13t[:, 0, 0:2])
            nc.scalar.dma_start(w13s[:, 0, 2:], w13t[:, 0, 2:])
            nc.scalar.dma_start(w13s[:, 1], w13t[:, 1])
            nc.scalar.dma_start(xss[0][:, 4:, :], xts[0][:, 4:, :])
            nc.scalar.dma_start(w13s[:, 4:5], w13t[:, 4:5])
            # sync: x chunk0 head, weight columns m2.., w2, x chunks 1.. (y
            # outputs are appended to this ring later, in program order).
            nc.sync.dma_start(xss[0][:, 0:2, :], xts[0][:, 0:2, :])
            nc.sync.dma_start(xss[0][:, 2:4, :], xts[0][:, 2:4, :])
            nc.sync.dma_start(w13s[:, 2], w13t[:, 2])
            nc.sync.dma_start(w13s[:, 3], w13t[:, 3])
            nc.sync.dma_start(w13s[:, 5:7], w13t[:, 5:7])
            nc.sync.dma_start(w13s[:, 7:], w13t[:, 7:])
            nc.sync.dma_start(w2s[:, 0:4], w2t[:, 0:4])
            nc.sync.dma_start(w2s[:, 4:], w2t[:, 4:])
            for j in range(1, len(chunks)):
                nc.sync.dma_start(xss[j][:], xts[j][:])

            # Warmup: tiny matmuls on a scratch tile keep the PE busy during
            # the DMA head so the HAM clock gate is at 8/8 when real work lands.
            nc.vector.memset(scratch[:], 0.0)
            wps = psW.tile([64, 64], dt.float32, name="warmps")
            for _ in range(WARM_N):
                nc.tensor.matmul(wps[:], scratch[:, 0:64], scratch[:], start=True, stop=True)

            c0 = 0
            for j, n in enumerate(chunks):
                xsj = xss[j]
                gs = gpool.tile([P, MT, n], dt.bfloat16, name="gs")
                for m in range(MT):
                    p1 = psA.tile([P, n], dt.float32, name="p1")
                    p3 = psA.tile([P, n], dt.float32, name="p3")
                    # Interleave the w1/w3 k-loops: halves the x k-tile
                    # consumption rate while the head DMAs are still landing.
                    for k in range(KT):
                        nc.tensor.matmul(
                            p1[:],
                            w13s[:, m, k, 0, :],
                            xsj[:, k, :],
                            start=(k == 0),
                            stop=(k == KT - 1),
                        )
                        nc.tensor.matmul(
                            p3[:],
                            w13s[:, m, k, 1, :],
                            xsj[:, k, :],
                            start=(k == 0),
                            stop=(k == KT - 1),
                        )
                    sil = spool.tile([P, n], dt.bfloat16, name="sil")
                    nc.scalar.activation(sil[:], p1[:], AF.Silu)
                    nc.vector.tensor_mul(gs[:, m, :], sil[:], p3[:])
                for i in range(KT):
                    py = psB.tile([P, n], dt.float32, name="py")
                    for m in range(MT):
                        nc.tensor.matmul(
                            py[:],
                            w2s[:, i, m * P:(m + 1) * P],
                            gs[:, m, :],
                            start=(m == 0),
                            stop=(m == MT - 1),
                        )
                    ys = ypool.tile([P, n], dt.bfloat16, name="ys")
                    nc.vector.tensor_copy(ys[:], py[:])
                    nc.sync.dma_start(yt[i, :, c0:c0 + n], ys[:])
                c0 += n

    nc.compile()
    return nc


def _get_nc(C):
    if C not in _NC_CACHE:
        _NC_CACHE[C] = _build_nc(C)
    return _NC_CACHE[C]


def _ensure_ntff_hook_importable():
    # bass_utils imports antenv.axon_hooks when tracing is requested (e.g. via
    # a BASS_TRACE env var); in containers whose antenv stub lacks that
    # submodule the import would crash. Register a null hook so tracing just
    # degrades to "no trace" instead.
    import sys
    import types

    try:
        import antenv.axon_hooks  # noqa: F401
    except ImportError:
        mod = types.ModuleType("antenv.axon_hooks")
        mod.get_axon_ntff_profile_hook = lambda: None
        mod.set_axon_ntff_profile_hook = lambda hook: None
        sys.modules["antenv.axon_hooks"] = mod


def kernel(x, expert_indices, w1, w2, w3):
    global LAST_RESULTS
    import os
    import sys

    # The bass kernel executes on the NeuronCores via the axon PJRT backend;
    # a JAX_PLATFORMS=cpu pin (commonly used for running jax reference code)
    # would hide those devices. Clear it if jax hasn't initialized yet.
    if os.environ.get("JAX_PLATFORMS") == "cpu" and "jax" not in sys.modules:
        del os.environ["JAX_PLATFORMS"]

    from concourse import bass_utils

    _ensure_ntff_hook_importable()
    x = np.asarray(x, dtype=np.float32)
    idx = np.asarray(expert_indices)
    w1 = np.asarray(w1, dtype=np.float32)
    w2 = np.asarray(w2, dtype=np.float32)
    w3 = np.asarray(w3, dtype=np.float32)

    bf16 = ml_dtypes.bfloat16

    # --- host routing: stable-sort the (token, k) slots by expert id ---
    flat = idx.reshape(-1).astype(np.int64)  # slot s = t*TOPK + k -> expert
    order = np.argsort(flat, kind="stable")  # slots grouped by expert
    counts = np.bincount(flat, minlength=E)
    starts = np.zeros(E + 1, dtype=np.int64)
    np.cumsum(counts, out=starts[1:])
    cmax = int(counts.max())
    C = max(512, -(-cmax // 16) * 16)  # pad capacity to a multiple of 16

    nc = _get_nc(C)

    chunks = _chunks_for(C)
    bounds = np.cumsum([0] + chunks)
    xb = x.astype(bf16)
    in_maps = []
    for e in range(E):
        slots = order[starts[e]:starts[e + 1]]
        tokens = slots // TOPK
        xg = np.zeros((C, DIM), dtype=bf16)
        xg[: len(tokens)] = xb[tokens]
        # [C, DIM] -> [P, KT, C] (partition-major), then per-chunk blocks
        xpkc = xg.T.reshape(KT, P, C).transpose(1, 0, 2)
        im = {
            f"xt{j}": np.ascontiguousarray(xpkc[:, :, bounds[j]:bounds[j + 1]])
            for j in range(len(chunks))
        }
        # w13t[p, m, k, s, j] = (w1 if s==0 else w3)[e][m*128+j, k*128+p]
        w1r = w1[e].astype(bf16).reshape(MT, P, KT, P).transpose(3, 0, 2, 1)
        w3r = w3[e].astype(bf16).reshape(MT, P, KT, P).transpose(3, 0, 2, 1)
        im["w13t"] = np.ascontiguousarray(np.stack([w1r, w3r], axis=3))
        # w2t[p, i, m*128+j] = w2[e][i*128+j, m*128+p]
        im["w2t"] = np.ascontiguousarray(
            w2[e].astype(bf16).reshape(KT, P, MT, P).transpose(3, 0, 2, 1)
        ).reshape(P, KT, MT * P)
        in_maps.append(im)

    res = bass_utils.run_bass_kernel_spmd(
        nc, in_maps, core_ids=list(range(NCORES)), trace=TRACE
    )
    LAST_RESULTS = res

    out = np.empty((T * TOPK, DIM), dtype=np.float32)
    for e in range(E):
        slots = order[starts[e]:starts[e + 1]]
        yt = res.results[e]["yt"]  # [KT, P, C] bf16
        y = yt.reshape(DIM, C).astype(np.float32)  # y.T
        out[slots] = y[:, : len(slots)].T
    return out.reshape(T, TOPK, DIM)


# revision 18
# speedup vs baseline: 1.1108x; 1.0410x over previous
"""Expert-parallel MoE SwiGLU FFN kernel for 8 Trainium2 NeuronCores.

Problem: T=4096 tokens, DIM=1024, E=8 experts, INTER=1408, top-2 routing.
Reference computes all experts densely then gathers; we instead route on the
host (sort token-slots by expert), assign one expert per core, and each core
runs a SwiGLU FFN over only its routed tokens (padded to a common capacity so
all 8 cores execute the same SPMD program).

Device layout (per core, everything "transposed" with tokens on the free dim):
  xt[j]  [P, KT, n]        bf16  x_gathered.T for chunk j (part, k-tile, token)
  w13t   [P, MT, 2, KT*P]  bf16  w1/w3 interleaved per m-column, p-major so
                                 any m-slice is per-partition contiguous
  w2t    [P, KT, MT*P]     bf16  w2.T, p-major, grouped by output k-tile so
                                 phase B can start once half has landed
  yt     [KT, P, C]        bf16  y.T tiled over DIM (output; host upcasts)

Compute per core and token chunk (n <= 512 for PSUM):
  h1.T = w1 @ x.T, h3.T = w3 @ x.T   interleaved per k so the head DMA feed
                                      of x k-tiles is half rate -> PSUM
  g.T  = silu(h1.T) * h3.T           -> SBUF bf16
  y.T  = w2 @ g.T                    -> PSUM -> SBUF bf16 -> HBM

Schedule notes (from perfetto traces): the matmul stream runs at the bf16
streaming roofline (LDWEIGHTS hidden), so the binding constraints are at the
edges. The head is DMA-bandwidth-bound (~370 GB/s/core aggregate), so input
transfers are issued in strict need-time order across the three trigger
queues (sync + scalar HWDGE, gpsimd SWDGE) with tiny first pieces; dummy
warmup matmuls during the DMA head flip the HAM clock gate (the PE otherwise
runs at 1.2 GHz for its first ~3.4us of activity).
"""

import numpy as np
import ml_dtypes

T, DIM, E, INTER, TOPK = 4096, 1024, 8, 1408, 2
NCORES = 8
P = 128
KT = DIM // P    # 8 k-tiles over DIM
MT = INTER // P  # 11 m-tiles over INTER

WARM_N = 68      # dummy warmup matmuls (N=64 each) to flip the HAM clock gate

TRACE = False  # test.py sets this to capture an NTFF profile
LAST_RESULTS = None  # BassKernelResults of the last run (for test.py)

_NC_CACHE = {}


def _chunks_for(C):
    # Split C into chunks of at most 512 (PSUM bank = 512 fp32), multiples of
    # 16, all >= 256 so matmuls stay stream-bound (LDWEIGHTS ~97ns hides only
    # when N >= ~233), smallest chunk first so compute starts earliest.
    nch = -(-C // 512)
    out = []
    rem = C
    for i in range(nch, 0, -1):
        n = min(512, -(-rem // i))
        n = -(-n // 16) * 16 if i > 1 else rem  # keep multiples of 16
        n = min(n, 512, rem)
        out.append(n)
        rem -= n
    out.sort()
    assert sum(out) == C and all(0 < n <= 512 for n in out), out
    return out


def _build_nc(C):
    import concourse.mybir as mybir
    import concourse.tile as tile
    from concourse import bacc

    dt = mybir.dt
    AF = mybir.ActivationFunctionType
    chunks = _chunks_for(C)

    nc = bacc.Bacc(
        "TRN2", target_bir_lowering=False, debug=False, enable_asserts=False
    )
    xts = [
        nc.dram_tensor(f"xt{j}", [P, KT, n], dt.bfloat16, kind="ExternalInput")
        for j, n in enumerate(chunks)
    ]
    w13t = nc.dram_tensor("w13t", [P, MT, KT, 2, P], dt.bfloat16, kind="ExternalInput")
    w2t = nc.dram_tensor("w2t", [P, KT, MT * P], dt.bfloat16, kind="ExternalInput")
    yt = nc.dram_tensor("yt", [KT, P, C], dt.bfloat16, kind="ExternalOutput")

    with tile.TileContext(nc) as tc:
        with (
            tc.tile_pool(name="persist", bufs=1) as wpool,
            tc.tile_pool(name="gbuf", bufs=3) as gpool,
            tc.tile_pool(name="ybuf", bufs=4) as ypool,
            tc.tile_pool(name="silbuf", bufs=3) as spool,
            tc.tile_pool(name="psA", bufs=2, space="PSUM") as psA,
            tc.tile_pool(name="psB", bufs=2, space="PSUM") as psB,
            tc.tile_pool(name="psW", bufs=1, space="PSUM") as psW,
        ):
            xss = [wpool.tile([P, KT, n], dt.bfloat16, name=f"xs{j}")
                   for j, n in enumerate(chunks)]
            w13s = wpool.tile([P, MT, KT, 2, P], dt.bfloat16)
            w2s = wpool.tile([P, KT, MT * P], dt.bfloat16)
            scratch = wpool.tile([P, 64], dt.bfloat16, name="warm")

            # Input DMAs in strict need-time order; the head is DMA-BW bound,
            # so nothing may run ahead of more urgently needed bytes, and the
            # crunch set (x chunk0 + w13 m0/m1) is balanced across the three
            # trigger queues (each trigger costs ~0.6us of sequencer time; a
            # queue's transfers complete in ring order).
            # DMA engines service descriptors round-robin across rings, so a
            # transfer's effective priority is its ring position: everything
            # goes on the two HWDGE rings in deadline order (lazy bytes BEHIND
            # urgent bytes), split so both rings carry similar early loads.
            # The gpsimd SWDGE ring stays empty (its big software-generated
            # descriptors hog engine time).
            nc.scalar.dma_start(w13s[:, 0, 0:2], w13t[:, 0, 0:2])
            nc.scalar.dma_start(w13s[:, 0, 2:4], w13t[:, 0, 2:4])
            nc.scalar.dma_start(w13s[:, 0, 4:], w13t[:, 0, 4:])
            # gpsimd (SWDGE): x chunk0 tail only.
            nc.gpsimd.dma_start(xss[0][:, 4:, :], xts[0][:, 4:, :])
            # sync: x chunk0 head, weight columns m1.., w2, x chunks 1.. (y
            # outputs are appended to this ring later, in program order).
            nc.sync.dma_start(xss[0][:, 0:2, :], xts[0][:, 0:2, :])
            nc.sync.dma_start(xss[0][:, 2:4, :], xts[0][:, 2:4, :])
            nc.sync.dma_start(w13s[:, 1], w13t[:, 1])
            nc.sync.dma_start(w13s[:, 2], w13t[:, 2])
            nc.sync.dma_start(w13s[:, 3], w13t[:, 3])
            nc.sync.dma_start(w13s[:, 4:7], w13t[:, 4:7])
            nc.sync.dma_start(w13s[:, 7:], w13t[:, 7:])
            nc.sync.dma_start(w2s[:, 0:4], w2t[:, 0:4])
            nc.sync.dma_start(w2s[:, 4:], w2t[:, 4:])
            for j in range(1, len(chunks)):
                nc.sync.dma_start(xss[j][:], xts[j][:])

            # Warmup: tiny matmuls on a scratch tile keep the PE busy during
            # the DMA head so the HAM clock gate is at 8/8 when real work lands.
            nc.vector.memset(scratch[:], 0.0)
            wps = psW.tile([64, 64], dt.float32, name="warmps")
            for _ in range(WARM_N):
                nc.tensor.matmul(wps[:], scratch[:, 0:64], scratch[:], start=True, stop=True)

            c0 = 0
            for j, n in enumerate(chunks):
                xsj = xss[j]
                gs = gpool.tile([P, MT, n], dt.bfloat16, name="gs")
                for m in range(MT):
                    p1 = psA.tile([P, n], dt.float32, name="p1")
                    p3 = psA.tile([P, n], dt.float32, name="p3")
                    # Interleave the w1/w3 k-loops: halves the x k-tile
                    # consumption rate while the head DMAs are still landing.
                    for k in range(KT):
                        nc.tensor.matmul(
                            p1[:],
                            w13s[:, m, k, 0, :],
                            xsj[:, k, :],
                            start=(k == 0),
                            stop=(k == KT - 1),
                        )
                        nc.tensor.matmul(
                            p3[:],
                            w13s[:, m, k, 1, :],
                            xsj[:, k, :],
                            start=(k == 0),
                            stop=(k == KT - 1),
                        )
                    sil = spool.tile([P, n], dt.bfloat16, name="sil")
                    nc.scalar.activation(sil[:], p1[:], AF.Silu)
                    nc.vector.tensor_mul(gs[:, m, :], sil[:], p3[:])
                for i in range(KT):
                    py = psB.tile([P, n], dt.float32, name="py")
                    for m in range(MT):
                        nc.tensor.matmul(
                            py[:],
                            w2s[:, i, m * P:(m + 1) * P],
                            gs[:, m, :],
                            start=(m == 0),
                            stop=(m == MT - 1),
                        )
                    ys = ypool.tile([P, n], dt.bfloat16, name="ys")
                    nc.vector.tensor_copy(ys[:], py[:])
                    nc.sync.dma_start(yt[i, :, c0:c0 + n], ys[:])
                c0 += n

    nc.compile()
    return nc


def _get_nc(C):
    if C not in _NC_CACHE:
        _NC_CACHE[C] = _build_nc(C)
    return _NC_CACHE[C]


def _ensure_ntff_hook_importable():
    # bass_utils imports antenv.axon_hooks when tracing is requested (e.g. via
    # a BASS_TRACE env var); in containers whose antenv stub lacks that
    # submodule the import would crash. Register a null hook so tracing just
    # degrades to "no trace" instead.
    import sys
    import types

    try:
        import antenv.axon_hooks  # noqa: F401
    except ImportError:
        mod = types.ModuleType("antenv.axon_hooks")
        mod.get_axon_ntff_profile_hook = lambda: None
        mod.set_axon_ntff_profile_hook = lambda hook: None
        sys.modules["antenv.axon_hooks"] = mod


def kernel(x, expert_indices, w1, w2, w3):
    global LAST_RESULTS
    import os
    import sys

    # The bass kernel executes on the NeuronCores via the axon PJRT backend;
    # a JAX_PLATFORMS=cpu pin (commonly used for running jax reference code)
    # would hide those devices. Clear it if jax hasn't initialized yet.
    if os.environ.get("JAX_PLATFORMS") == "cpu" and "jax" not in sys.modules:
        del os.environ["JAX_PLATFORMS"]

    from concourse import bass_utils

    _ensure_ntff_hook_importable()
    x = np.asarray(x, dtype=np.float32)
    idx = np.asarray(expert_indices)
    w1 = np.asarray(w1, dtype=np.float32)
    w2 = np.asarray(w2, dtype=np.float32)
    w3 = np.asarray(w3, dtype=np.float32)

    bf16 = ml_dtypes.bfloat16

    # --- host routing: stable-sort the (token, k) slots by expert id ---
    flat = idx.reshape(-1).astype(np.int64)  # slot s = t*TOPK + k -> expert
    order = np.argsort(flat, kind="stable")  # slots grouped by expert
    counts = np.bincount(flat, minlength=E)
    starts = np.zeros(E + 1, dtype=np.int64)
    np.cumsum(counts, out=starts[1:])
    cmax = int(counts.max())
    C = max(512, -(-cmax // 16) * 16)  # pad capacity to a multiple of 16

    nc = _get_nc(C)

    chunks = _chunks_for(C)
    bounds = np.cumsum([0] + chunks)
    xb = x.astype(bf16)
    in_maps = []
    for e in range(E):
        slots = order[starts[e]:starts[e + 1]]
        tokens = slots // TOPK
        xg = np.zeros((C, DIM), dtype=bf16)
        xg[: len(tokens)] = xb[tokens]
        # [C, DIM] -> [P, KT, C] (partition-major), then per-chunk blocks
        xpkc = xg.T.reshape(KT, P, C).transpose(1, 0, 2)
        im = {
            f"xt{j}": np.ascontiguousarray(xpkc[:, :, bounds[j]:bounds[j + 1]])
            for j in range(len(chunks))
        }
        # w13t[p, m, k, s, j] = (w1 if s==0 else w3)[e][m*128+j, k*128+p]
        w1r = w1[e].astype(bf16).reshape(MT, P, KT, P).transpose(3, 0, 2, 1)
        w3r = w3[e].astype(bf16).reshape(MT, P, KT, P).transpose(3, 0, 2, 1)
        im["w13t"] = np.ascontiguousarray(np.stack([w1r, w3r], axis=3))
        # w2t[p, i, m*128+j] = w2[e][i*128+j, m*128+p]
        im["w2t"] = np.ascontiguousarray(
            w2[e].astype(bf16).reshape(KT, P, MT, P).transpose(3, 0, 2, 1)
        ).reshape(P, KT, MT * P)
        in_maps.append(im)

    res = bass_utils.run_bass_kernel_spmd(
        nc, in_maps, core_ids=list(range(NCORES)), trace=TRACE
    )
    LAST_RESULTS = res

    out = np.empty((T * TOPK, DIM), dtype=np.float32)
    for e in range(E):
        slots = order[starts[e]:starts[e + 1]]
        yt = res.results[e]["yt"]  # [KT, P, C] bf16
        y = yt.reshape(DIM, C).astype(np.float32)  # y.T
        out[slots] = y[:, : len(slots)].T
    return out.reshape(T, TOPK, DIM)
